# revision 2
# baseline (speedup 1.0000x reference)
"""LoG kernel v3: v2 DMA/geometry + plain fp32 matmuls (fp32r rounds operands
too coarsely: rel err 0.76). Compact bands since fp32 is 4 cyc/row at any N.

v2 redesign, driven by profiling the v1 baseline (501 us):
  1. All DMAs are exactly 128 partitions. Mixed partition counts (113-row
     halo loads vs 128-row stores) collapse HWDGE descriptor spreading onto
     a single SDMA engine (~25 GB/s); uniform 128-row DMAs spread across
     all 16 engines (~300 GB/s measured).
  2. All matmuls have out free-dim >= 256: fp32r streams 1 cycle/row only
     when the moving dim is >= 256, else 4 cycles/row. Stage-1 packs
     [A-band | B-band | zero pad] into one 256-wide rhs; stage-2 embeds
     each chunk's band into a 256/512-wide zero-padded rhs, all chunks
     accumulating into one PSUM bank.
  3. Clip epilogue: ScalarE Relu(psum + 1) -> contiguous tmp, then one
     strided DVE tensor_scalar_min(255) interleaving into the NHWC out
     tile. (v1 burned 385 us on GpSimd strided min.)

Math (as v1): both depthwise convs are separable/symmetric and compose into
an 11x11 separable pair: out = clip(Bx(Ay x) + Ax(By x) + 1, 0, 255) with
A = gauss3 (*) SMOOTH_9, B = gauss3 (*) D2_9 (11 taps each), reflect-101
folded into per-chunk band matrices.

Geometry (shared by y and x axes): 5 input windows of exactly 128 rows
  W = [0,128) [93,221) [191,319) [289,417) [384,512)
and 5 output chunks
  C = [0,98) [98,196) [196,294) [294,392) [392,512)
with chunk j's 11-tap support contained in window j (reflect at edges).

Stage 1 (y-conv, data-stationary, output transposed):
  psum[x_win_i 128, 256*t] = x[y_win_j, x_win_i].T @ [A_j|B_j|pad]
  two y-chunks share one PSUM bank; one 4D-AP copy drains u,v planes.
Stage 2 (x-conv, data-stationary, back to image orientation):
  psum[y_blk 128, 512] += u_win_j.T @ embB_j  +  v_win_j.T @ embA_j
  (first MM is 512-wide with start=True to clear the bank; the rest are
  256-wide embeds at col offsets 0,0,38,256,256).
"""

import numpy as np

N_CORES = 8
BATCH = 32
IMG_PER_CORE = BATCH // N_CORES
H = W = 512
C = 3
RAD = 5  # half width of composed 11-tap filter
WIN = 128  # all input windows exactly 128 rows (DMA + matmul K alignment)

# window starts and output chunk bounds (see module docstring)
WSTARTS = [0, 93, 191, 289, 384]
CBOUNDS = [0, 98, 196, 294, 392, 512]
NCHUNK = 5
# stage-2 rhs embedding: (col_base, width) per chunk; chunk 0 is 512 wide
# and runs first with start=True to clear the whole PSUM bank.
EMBED = [(CBOUNDS[j], CBOUNDS[j + 1] - CBOUNDS[j]) for j in range(5)]


def _chunks():
    out = []
    for j in range(NCHUNK):
        s, e = CBOUNDS[j], CBOUNDS[j + 1]
        out.append((s, e, WSTARTS[j]))
    return out


def make_taps():
    g = np.exp(-((np.arange(3) - 1.0) ** 2) / 2.0)
    g = g / g.sum()
    S = np.array([1, 8, 28, 56, 70, 56, 28, 8, 1], dtype=np.float64)
    D2 = np.array([1, 4, 4, -4, -10, -4, 4, 4, 1], dtype=np.float64)
    return np.convolve(g, S), np.convolve(g, D2)


def _reflect(i, n):
    if i < 0:
        return -i
    if i > n - 1:
        return 2 * (n - 1) - i
    return i


def make_bands1():
    """Stage-1 bands: per chunk j a [128, 256] tile = [A_j | B_j | zeros]."""
    A, B = make_taps()
    bands = []
    for s, e, w0 in _chunks():
        nj = e - s
        band = np.zeros((WIN, 2 * (e - s)), np.float64)
        for t in range(nj):
            y = s + t
            for off in range(-RAD, RAD + 1):
                r = _reflect(y + off, H) - w0
                band[r, t] += A[off + RAD]
                band[r, nj + t] += B[off + RAD]
        bands.append(np.ascontiguousarray(band, np.float32))
    return bands


def make_bands2():
    """Stage-2 embedded bands: per chunk j two tiles [128, W_j] (B for u,
    A for v) with the chunk's band at cols [s_j - base_j, e_j - base_j)."""
    A, B = make_taps()
    bu, bv = [], []
    for j, (s, e, w0) in enumerate(_chunks()):
        base, width = EMBED[j]
        tb = np.zeros((WIN, width), np.float64)
        ta = np.zeros((WIN, width), np.float64)
        for x in range(s, e):
            for off in range(-RAD, RAD + 1):
                r = _reflect(x + off, W) - w0
                tb[r, x - base] += B[off + RAD]
                ta[r, x - base] += A[off + RAD]
        bu.append(np.ascontiguousarray(tb, np.float32))
        bv.append(np.ascontiguousarray(ta, np.float32))
    return bu, bv


def _pairs(n):
    ps, i = [], 0
    while i < n:
        ps.append(tuple(range(i, min(i + 2, n))))
        i += 2
    return ps


def build_bass(n_imgs=IMG_PER_CORE, h=H, w=W, c=C):
    import concourse.bacc as bacc
    import concourse.mybir as mybir
    import concourse.tile as tile

    f32 = mybir.dt.float32
    f32r = mybir.dt.float32r
    relu = mybir.ActivationFunctionType.Relu
    chunks = _chunks()
    jpairs = _pairs(NCHUNK)
    n_yblk = h // 128

    nc = bacc.Bacc("TRN2", target_bir_lowering=False, debug=False)
    x_d = nc.dram_tensor("x", [n_imgs, h, w, c], f32, kind="ExternalInput")
    out_d = nc.dram_tensor("out", [n_imgs, h, w, c], f32, kind="ExternalOutput")
    b1_d = [
        nc.dram_tensor(f"b1_{j}", [WIN, 2 * (CBOUNDS[j + 1] - CBOUNDS[j])], f32, kind="ExternalInput")
        for j in range(NCHUNK)
    ]
    bu_d = [
        nc.dram_tensor(f"bu_{j}", [WIN, EMBED[j][1]], f32, kind="ExternalInput")
        for j in range(NCHUNK)
    ]
    bv_d = [
        nc.dram_tensor(f"bv_{j}", [WIN, EMBED[j][1]], f32, kind="ExternalInput")
        for j in range(NCHUNK)
    ]

    with tile.TileContext(nc) as tc:
        with (
            tc.tile_pool(name="const", bufs=1) as cpool,
            tc.tile_pool(name="xin", bufs=2) as xpool,
            tc.tile_pool(name="uv", bufs=2) as uvpool,
            tc.tile_pool(name="tmp", bufs=2) as tpool,
            tc.tile_pool(name="outp", bufs=2) as opool,
            tc.tile_pool(name="ps", bufs=3, space="PSUM") as pspool,
            tc.tile_pool(name="pso", bufs=3, space="PSUM") as psopool,
        ):
            band1, bandu, bandv = [], [], []
            for j in range(NCHUNK):
                tb = cpool.tile([WIN, 2 * (CBOUNDS[j + 1] - CBOUNDS[j])], f32, name=f"b1_{j}")
                nc.sync.dma_start(tb[:], b1_d[j].ap())
                band1.append(tb)
                tu = cpool.tile([WIN, EMBED[j][1]], f32, name=f"bu_{j}")
                nc.sync.dma_start(tu[:], bu_d[j].ap())
                bandu.append(tu)
                tv = cpool.tile([WIN, EMBED[j][1]], f32, name=f"bv_{j}")
                nc.sync.dma_start(tv[:], bv_d[j].ap())
                bandv.append(tv)

            for n in range(n_imgs):
                xrows = []
                for j in range(NCHUNK):
                    w0 = WSTARTS[j]
                    xr = xpool.tile([WIN, w, c], f32, tag=f"x{j}", name=f"x{j}_{n}")
                    nc.sync.dma_start(xr[:], x_d.ap()[n, w0 : w0 + WIN, :, :])
                    xrows.append(xr)
                outs = []
                for b in range(n_yblk):
                    ot = opool.tile([128, w, c], f32, tag=f"o{b}", name=f"o{b}_{n}")
                    outs.append(ot)
                for ci in range(c):
                    # stage 1: y-conv, transposed output per x-window
                    uvts = []
                    for i in range(NCHUNK):
                        uvt = uvpool.tile(
                            [WIN, 2, h], f32, tag=f"uv{i}", name=f"uv{i}_{n}_{ci}"
                        )
                        uvts.append(uvt)
                    for jp in jpairs:
                        nj = chunks[jp[0]][1] - chunks[jp[0]][0]
                        sj0 = chunks[jp[0]][0]
                        for i in range(NCHUNK):
                            wi = WSTARTS[i]
                            ps = pspool.tile([WIN, 512], f32, tag="ps")
                            for t, j in enumerate(jp):
                                lhsT = xrows[j][:, wi : wi + WIN, ci]
                                nc.tensor.matmul(
                                    ps[:, t * 2 * nj : (t + 1) * 2 * nj],
                                    lhsT,
                                    band1[j][:],
                                    start=True,
                                    stop=True,
                                )
                            # drain u,v (skip zero pad cols) with one 4D copy
                            if len(jp) > 1:
                                src = ps[:, 0 : len(jp) * 2 * nj].rearrange(
                                    "m (js uv x) -> m uv js x", js=len(jp), uv=2
                                )
                                dst = uvts[i][
                                    :, :, sj0 : sj0 + len(jp) * nj
                                ].rearrange("m uv (js x) -> m uv js x", js=len(jp))
                            else:
                                src = ps[:, 0 : 2 * nj].rearrange(
                                    "m (uv x) -> m uv x", uv=2
                                )
                                dst = uvts[i][:, :, sj0 : sj0 + nj]
                            if i % 2 == 0:
                                nc.vector.tensor_copy(dst, src)
                            else:
                                nc.scalar.copy(dst, src)
                    # stage 2: x-conv back to image orientation + clip
                    for b in range(n_yblk):
                        pso = psopool.tile([128, 512], f32, tag="pso")
                        for j in range(NCHUNK):
                            base, width = EMBED[j]
                            nc.tensor.matmul(
                                pso[:, base : base + width],
                                uvts[j][:, 0, b * 128 : (b + 1) * 128],
                                bandu[j][:],
                                start=True,
                                stop=False,
                            )
                            nc.tensor.matmul(
                                pso[:, base : base + width],
                                uvts[j][:, 1, b * 128 : (b + 1) * 128],
                                bandv[j][:],
                                start=False,
                                stop=True,
                            )
                        tmp = tpool.tile([128, 512], f32, tag=f"t{b}", name=f"t{b}_{n}_{ci}")
                        nc.scalar.activation(tmp[:], pso[:], relu, bias=1.0)
                        nc.vector.tensor_scalar_min(outs[b][:, :, ci], tmp[:], 255.0)
                for b in range(n_yblk):
                    nc.sync.dma_start(
                        out_d.ap()[n, b * 128 : (b + 1) * 128, :, :], outs[b][:]
                    )

    nc.compile()
    return nc


_CACHE = {}


def _get_nc():
    if "nc" not in _CACHE:
        _CACHE["nc"] = build_bass()
    return _CACHE["nc"]


def kernel(x: np.ndarray) -> np.ndarray:
    from concourse import bass_utils

    nc = _get_nc()
    b1 = make_bands1()
    bu, bv = make_bands2()
    const_map = {f"b1_{j}": b1[j] for j in range(NCHUNK)}
    const_map.update({f"bu_{j}": bu[j] for j in range(NCHUNK)})
    const_map.update({f"bv_{j}": bv[j] for j in range(NCHUNK)})
    x = np.ascontiguousarray(x, dtype=np.float32)
    in_maps = [
        {"x": x[k * IMG_PER_CORE : (k + 1) * IMG_PER_CORE], **const_map}
        for k in range(N_CORES)
    ]
    res = bass_utils.run_bass_kernel_spmd(nc, in_maps, core_ids=list(range(N_CORES)))
    _CACHE["last_result"] = res
    out = np.concatenate([r["out"] for r in res.results], axis=0)
    return out.astype(np.float32)


# revision 3
# speedup vs baseline: 1.2236x; 1.2236x over previous
"""LoG kernel v3: v2 DMA/geometry + plain fp32 matmuls (fp32r rounds operands
too coarsely: rel err 0.76). Compact bands since fp32 is 4 cyc/row at any N.

v2 redesign, driven by profiling the v1 baseline (501 us):
  1. All DMAs are exactly 128 partitions. Mixed partition counts (113-row
     halo loads vs 128-row stores) collapse HWDGE descriptor spreading onto
     a single SDMA engine (~25 GB/s); uniform 128-row DMAs spread across
     all 16 engines (~300 GB/s measured).
  2. All matmuls have out free-dim >= 256: fp32r streams 1 cycle/row only
     when the moving dim is >= 256, else 4 cycles/row. Stage-1 packs
     [A-band | B-band | zero pad] into one 256-wide rhs; stage-2 embeds
     each chunk's band into a 256/512-wide zero-padded rhs, all chunks
     accumulating into one PSUM bank.
  3. Clip epilogue: ScalarE Relu(psum + 1) -> contiguous tmp, then one
     strided DVE tensor_scalar_min(255) interleaving into the NHWC out
     tile. (v1 burned 385 us on GpSimd strided min.)

Math (as v1): both depthwise convs are separable/symmetric and compose into
an 11x11 separable pair: out = clip(Bx(Ay x) + Ax(By x) + 1, 0, 255) with
A = gauss3 (*) SMOOTH_9, B = gauss3 (*) D2_9 (11 taps each), reflect-101
folded into per-chunk band matrices.

Geometry (shared by y and x axes): 5 input windows of exactly 128 rows
  W = [0,128) [93,221) [191,319) [289,417) [384,512)
and 5 output chunks
  C = [0,98) [98,196) [196,294) [294,392) [392,512)
with chunk j's 11-tap support contained in window j (reflect at edges).

Stage 1 (y-conv, data-stationary, output transposed):
  psum[x_win_i 128, 256*t] = x[y_win_j, x_win_i].T @ [A_j|B_j|pad]
  two y-chunks share one PSUM bank; one 4D-AP copy drains u,v planes.
Stage 2 (x-conv, data-stationary, back to image orientation):
  psum[y_blk 128, 512] += u_win_j.T @ embB_j  +  v_win_j.T @ embA_j
  (first MM is 512-wide with start=True to clear the bank; the rest are
  256-wide embeds at col offsets 0,0,38,256,256).
"""

import numpy as np

N_CORES = 8
BATCH = 32
IMG_PER_CORE = BATCH // N_CORES
H = W = 512
C = 3
RAD = 5  # half width of composed 11-tap filter
WIN = 128  # all input windows exactly 128 rows (DMA + matmul K alignment)

# window starts and output chunk bounds (see module docstring)
WSTARTS = [0, 93, 191, 289, 384]
CBOUNDS = [0, 98, 196, 294, 392, 512]
NCHUNK = 5
# stage-2 rhs embedding: (col_base, width) per chunk; chunk 0 is 512 wide
# and runs first with start=True to clear the whole PSUM bank.
EMBED = [(CBOUNDS[j], CBOUNDS[j + 1] - CBOUNDS[j]) for j in range(5)]


def _chunks():
    out = []
    for j in range(NCHUNK):
        s, e = CBOUNDS[j], CBOUNDS[j + 1]
        out.append((s, e, WSTARTS[j]))
    return out


def make_taps():
    g = np.exp(-((np.arange(3) - 1.0) ** 2) / 2.0)
    g = g / g.sum()
    S = np.array([1, 8, 28, 56, 70, 56, 28, 8, 1], dtype=np.float64)
    D2 = np.array([1, 4, 4, -4, -10, -4, 4, 4, 1], dtype=np.float64)
    return np.convolve(g, S), np.convolve(g, D2)


def _reflect(i, n):
    if i < 0:
        return -i
    if i > n - 1:
        return 2 * (n - 1) - i
    return i


def make_bands1():
    """Stage-1 bands: per chunk j a [128, 256] tile = [A_j | B_j | zeros]."""
    A, B = make_taps()
    bands = []
    for s, e, w0 in _chunks():
        nj = e - s
        band = np.zeros((WIN, 2 * (e - s)), np.float64)
        for t in range(nj):
            y = s + t
            for off in range(-RAD, RAD + 1):
                r = _reflect(y + off, H) - w0
                band[r, t] += A[off + RAD]
                band[r, nj + t] += B[off + RAD]
        bands.append(np.ascontiguousarray(band, np.float32))
    return bands


def make_bands2():
    """Stage-2 embedded bands: per chunk j two tiles [128, W_j] (B for u,
    A for v) with the chunk's band at cols [s_j - base_j, e_j - base_j)."""
    A, B = make_taps()
    bu, bv = [], []
    for j, (s, e, w0) in enumerate(_chunks()):
        base, width = EMBED[j]
        tb = np.zeros((WIN, width), np.float64)
        ta = np.zeros((WIN, width), np.float64)
        for x in range(s, e):
            for off in range(-RAD, RAD + 1):
                r = _reflect(x + off, W) - w0
                tb[r, x - base] += B[off + RAD]
                ta[r, x - base] += A[off + RAD]
        bu.append(np.ascontiguousarray(tb, np.float32))
        bv.append(np.ascontiguousarray(ta, np.float32))
    return bu, bv


def _pairs(n):
    ps, i = [], 0
    while i < n:
        ps.append(tuple(range(i, min(i + 2, n))))
        i += 2
    return ps


def build_bass(n_imgs=IMG_PER_CORE, h=H, w=W, c=C):
    import concourse.bacc as bacc
    import concourse.mybir as mybir
    import concourse.tile as tile

    f32 = mybir.dt.float32
    f32r = mybir.dt.float32r
    relu = mybir.ActivationFunctionType.Relu
    chunks = _chunks()
    jpairs = _pairs(NCHUNK)
    n_yblk = h // 128

    nc = bacc.Bacc("TRN2", target_bir_lowering=False, debug=False)
    x_d = nc.dram_tensor("x", [n_imgs, h, w, c], f32, kind="ExternalInput")
    out_d = nc.dram_tensor("out", [n_imgs, h, w, c], f32, kind="ExternalOutput")
    b1_d = [
        nc.dram_tensor(f"b1_{j}", [WIN, 2 * (CBOUNDS[j + 1] - CBOUNDS[j])], f32, kind="ExternalInput")
        for j in range(NCHUNK)
    ]
    bu_d = [
        nc.dram_tensor(f"bu_{j}", [WIN, EMBED[j][1]], f32, kind="ExternalInput")
        for j in range(NCHUNK)
    ]
    bv_d = [
        nc.dram_tensor(f"bv_{j}", [WIN, EMBED[j][1]], f32, kind="ExternalInput")
        for j in range(NCHUNK)
    ]

    with tile.TileContext(nc) as tc:
        with (
            tc.tile_pool(name="const", bufs=1) as cpool,
            tc.tile_pool(name="xin", bufs=2) as xpool,
            tc.tile_pool(name="uv", bufs=2) as uvpool,
            tc.tile_pool(name="tmp", bufs=2) as tpool,
            tc.tile_pool(name="outp", bufs=2) as opool,
            tc.tile_pool(name="ps", bufs=4, space="PSUM") as pspool,
            tc.tile_pool(name="pso", bufs=4, space="PSUM") as psopool,
        ):
            band1, bandu, bandv = [], [], []
            for j in range(NCHUNK):
                tb = cpool.tile([WIN, 2 * (CBOUNDS[j + 1] - CBOUNDS[j])], f32, name=f"b1_{j}")
                nc.sync.dma_start(tb[:], b1_d[j].ap())
                band1.append(tb)
                tu = cpool.tile([WIN, EMBED[j][1]], f32, name=f"bu_{j}")
                nc.sync.dma_start(tu[:], bu_d[j].ap())
                bandu.append(tu)
                tv = cpool.tile([WIN, EMBED[j][1]], f32, name=f"bv_{j}")
                nc.sync.dma_start(tv[:], bv_d[j].ap())
                bandv.append(tv)

            for n in range(n_imgs):
                xrows = []
                for j in range(NCHUNK):
                    w0 = WSTARTS[j]
                    xr = xpool.tile([WIN, w, c], f32, tag=f"x{j}", name=f"x{j}_{n}")
                    nc.sync.dma_start(xr[:], x_d.ap()[n, w0 : w0 + WIN, :, :])
                    xrows.append(xr)
                outs = []
                for b in range(n_yblk):
                    ot = opool.tile([128, w, c], f32, tag=f"o{b}", name=f"o{b}_{n}")
                    outs.append(ot)
                for ci in range(c):
                    # stage 1: y-conv, transposed output per x-window
                    uvts = []
                    for i in range(NCHUNK):
                        uvt = uvpool.tile(
                            [WIN, 2, h], f32, tag=f"uv{i}", name=f"uv{i}_{n}_{ci}"
                        )
                        uvts.append(uvt)
                    for jp in jpairs:
                        nj = chunks[jp[0]][1] - chunks[jp[0]][0]
                        sj0 = chunks[jp[0]][0]
                        for i in range(NCHUNK):
                            wi = WSTARTS[i]
                            ps = pspool.tile([WIN, 512], f32, tag="ps")
                            for t, j in enumerate(jp):
                                lhsT = xrows[j][:, wi : wi + WIN, ci]
                                nc.tensor.matmul(
                                    ps[:, t * 2 * nj : (t + 1) * 2 * nj],
                                    lhsT,
                                    band1[j][:],
                                    start=True,
                                    stop=True,
                                )
                            # drain u,v (skip zero pad cols) with one 4D copy
                            if len(jp) > 1:
                                src = ps[:, 0 : len(jp) * 2 * nj].rearrange(
                                    "m (js uv x) -> m uv js x", js=len(jp), uv=2
                                )
                                dst = uvts[i][
                                    :, :, sj0 : sj0 + len(jp) * nj
                                ].rearrange("m uv (js x) -> m uv js x", js=len(jp))
                            else:
                                src = ps[:, 0 : 2 * nj].rearrange(
                                    "m (uv x) -> m uv x", uv=2
                                )
                                dst = uvts[i][:, :, sj0 : sj0 + nj]
                            if i % 2 == 0:
                                nc.vector.tensor_copy(dst, src)
                            else:
                                nc.scalar.copy(dst, src)
                    # stage 2: x-conv back to image orientation + clip
                    for b in range(n_yblk):
                        pso = psopool.tile([128, 512], f32, tag="pso")
                        for j in range(NCHUNK):
                            base, width = EMBED[j]
                            nc.tensor.matmul(
                                pso[:, base : base + width],
                                uvts[j][:, 0, b * 128 : (b + 1) * 128],
                                bandu[j][:],
                                start=True,
                                stop=False,
                            )
                            nc.tensor.matmul(
                                pso[:, base : base + width],
                                uvts[j][:, 1, b * 128 : (b + 1) * 128],
                                bandv[j][:],
                                start=False,
                                stop=True,
                            )
                        tmp = tpool.tile([128, 512], f32, tag=f"t{b}", name=f"t{b}_{n}_{ci}")
                        nc.scalar.activation(tmp[:], pso[:], relu, bias=1.0)
                        nc.vector.tensor_scalar_min(outs[b][:, :, ci], tmp[:], 255.0)
                for b in range(n_yblk):
                    nc.sync.dma_start(
                        out_d.ap()[n, b * 128 : (b + 1) * 128, :, :], outs[b][:]
                    )

    nc.compile()
    return nc


_CACHE = {}


def _get_nc():
    if "nc" not in _CACHE:
        _CACHE["nc"] = build_bass()
    return _CACHE["nc"]


def kernel(x: np.ndarray) -> np.ndarray:
    from concourse import bass_utils

    nc = _get_nc()
    b1 = make_bands1()
    bu, bv = make_bands2()
    const_map = {f"b1_{j}": b1[j] for j in range(NCHUNK)}
    const_map.update({f"bu_{j}": bu[j] for j in range(NCHUNK)})
    const_map.update({f"bv_{j}": bv[j] for j in range(NCHUNK)})
    x = np.ascontiguousarray(x, dtype=np.float32)
    in_maps = [
        {"x": x[k * IMG_PER_CORE : (k + 1) * IMG_PER_CORE], **const_map}
        for k in range(N_CORES)
    ]
    res = bass_utils.run_bass_kernel_spmd(nc, in_maps, core_ids=list(range(N_CORES)))
    _CACHE["last_result"] = res
    out = np.concatenate([r["out"] for r in res.results], axis=0)
    return out.astype(np.float32)


# revision 4
# speedup vs baseline: 1.3440x; 1.0984x over previous
"""LoG kernel v4: fp16 hi/lo compensated split -> all matmuls at 1 cycle/row.

v3 (fp32 matmuls, 4 cyc/row) is Tensor-bound at ~226 us PE busy. fp16 MMs
stream 1 cyc/row at any N and get FWL (4x faster LDWEIGHTS), but naive fp16
fails: the composed filter's gain (sqrt(sum k^2) ~ 2563) amplifies operand
rounding. Fix: first-order error-compensated splits of BOTH the data and the
bands:  p*q ~= p_hi*q_hi + p_hi*q_lo + p_lo*q_hi  (p_lo*q_lo dropped).
  stage 1: 3 MMs per (x-window, y-chunk):  xh*b1h, xh*b1l, xl*b1h
  stage 2: 6 MMs per (y-blk, x-chunk):     uh*Bh, uh*Bl, ul*Bh, vh*Ah, vh*Al, vl*Ah
Splits: x_hi=fp16(x), x_lo=fp16(x-x_hi) computed on ScalarE/DVE per image;
u_hi=fp16(psum), u_lo=fp16(psum-u_hi) in the stage-1 drain (2 passes).
Residual error ~0.3 rms (fp16 max 65504 just fits u <= 255*256 = 65280).

Geometry + DMA rules identical to v3 (five 128-row windows per axis; all
DMAs exactly 128 partitions; chunk psum slices disjoint).
"""

import numpy as np

N_CORES = 8
BATCH = 32
IMG_PER_CORE = BATCH // N_CORES
H = W = 512
C = 3
RAD = 5
WIN = 128

WSTARTS = [0, 93, 191, 289, 384]
CBOUNDS = [0, 98, 196, 294, 392, 512]
NCHUNK = 5


def _chunks():
    return [(CBOUNDS[j], CBOUNDS[j + 1], WSTARTS[j]) for j in range(NCHUNK)]


def make_taps():
    g = np.exp(-((np.arange(3) - 1.0) ** 2) / 2.0)
    g = g / g.sum()
    S = np.array([1, 8, 28, 56, 70, 56, 28, 8, 1], dtype=np.float64)
    D2 = np.array([1, 4, 4, -4, -10, -4, 4, 4, 1], dtype=np.float64)
    return np.convolve(g, S), np.convolve(g, D2)


def _reflect(i, n):
    if i < 0:
        return -i
    if i > n - 1:
        return 2 * (n - 1) - i
    return i


def _split16(a):
    hi = a.astype(np.float16)
    lo = (a - hi.astype(np.float64)).astype(np.float16)
    return hi, lo


def make_bands1():
    """Per chunk j: [128, 2nj] = [A_j | B_j], split into fp16 hi/lo."""
    A, B = make_taps()
    his, los = [], []
    for s, e, w0 in _chunks():
        nj = e - s
        band = np.zeros((WIN, 2 * nj), np.float64)
        for t in range(nj):
            for off in range(-RAD, RAD + 1):
                r = _reflect(s + t + off, H) - w0
                band[r, t] += A[off + RAD]
                band[r, nj + t] += B[off + RAD]
        hi, lo = _split16(band)
        his.append(hi)
        los.append(lo)
    return his, los


def make_bands2():
    """Per chunk j: [128, nj] B-band (for u) and A-band (for v), fp16 hi/lo."""
    A, B = make_taps()
    out = {"buh": [], "bul": [], "bvh": [], "bvl": []}
    for s, e, w0 in _chunks():
        nj = e - s
        tb = np.zeros((WIN, nj), np.float64)
        ta = np.zeros((WIN, nj), np.float64)
        for t in range(nj):
            for off in range(-RAD, RAD + 1):
                r = _reflect(s + t + off, W) - w0
                tb[r, t] += B[off + RAD]
                ta[r, t] += A[off + RAD]
        bh, bl = _split16(tb)
        ah, al = _split16(ta)
        out["buh"].append(bh)
        out["bul"].append(bl)
        out["bvh"].append(ah)
        out["bvl"].append(al)
    return out


def _pairs(n):
    ps, i = [], 0
    while i < n:
        ps.append(tuple(range(i, min(i + 2, n))))
        i += 2
    return ps


def build_bass(n_imgs=IMG_PER_CORE, h=H, w=W, c=C):
    import concourse.bacc as bacc
    import concourse.mybir as mybir
    import concourse.tile as tile

    f32 = mybir.dt.float32
    f16 = mybir.dt.float16
    relu = mybir.ActivationFunctionType.Relu
    chunks = _chunks()
    jpairs = _pairs(NCHUNK)
    n_yblk = h // 128

    nc = bacc.Bacc("TRN2", target_bir_lowering=False, debug=False)
    x_d = nc.dram_tensor("x", [n_imgs, h, w, c], f32, kind="ExternalInput")
    out_d = nc.dram_tensor("out", [n_imgs, h, w, c], f32, kind="ExternalOutput")
    njs = [e - s for s, e, _ in chunks]
    band_names = (
        [(f"b1h_{j}", 2 * njs[j]) for j in range(NCHUNK)]
        + [(f"b1l_{j}", 2 * njs[j]) for j in range(NCHUNK)]
        + [(f"{k}_{j}", njs[j]) for k in ("buh", "bul", "bvh", "bvl") for j in range(NCHUNK)]
    )
    band_d = {
        name: nc.dram_tensor(name, [WIN, width], f16, kind="ExternalInput")
        for name, width in band_names
    }

    with tile.TileContext(nc) as tc:
        with (
            tc.tile_pool(name="const", bufs=1) as cpool,
            tc.tile_pool(name="xin", bufs=2) as xpool,
            tc.tile_pool(name="xhl", bufs=2) as xhlpool,
            tc.tile_pool(name="uv", bufs=2) as uvpool,
            tc.tile_pool(name="outp", bufs=2) as opool,
            tc.tile_pool(name="ps", bufs=4, space="PSUM") as pspool,
            tc.tile_pool(name="pso", bufs=4, space="PSUM") as psopool,
        ):
            band = {}
            for name, width in band_names:
                tb = cpool.tile([WIN, width], f16, name=name)
                nc.sync.dma_start(tb[:], band_d[name].ap())
                band[name] = tb

            for n in range(n_imgs):
                xhls = []
                for j in range(NCHUNK):
                    w0 = WSTARTS[j]
                    xr = xpool.tile([WIN, w, c], f32, tag=f"xf{j % 2}", name=f"x{j}_{n}")
                    nc.sync.dma_start(xr[:], x_d.ap()[n, w0 : w0 + WIN, :, :])
                    xhl = xhlpool.tile([WIN, 2, w, c], f16, tag=f"xhl{j}", name=f"xhl{j}_{n}")
                    # split: hi = fp16(x) on ScalarE, lo = fp16(x - hi) on DVE
                    nc.scalar.copy(xhl[:, 0], xr[:])
                    nc.vector.tensor_sub(xhl[:, 1], xr[:], xhl[:, 0])
                    xhls.append(xhl)
                outs = []
                for b in range(n_yblk):
                    ot = opool.tile([128, w, c], f32, tag=f"o{b}", name=f"o{b}_{n}")
                    outs.append(ot)
                for ci in range(c):
                    # stage 1: y-conv, transposed output per x-window
                    uvts = []
                    for i in range(NCHUNK):
                        uvt = uvpool.tile(
                            [WIN, 2, 2, h], f16, tag=f"uv{i}", name=f"uv{i}_{n}_{ci}"
                        )
                        uvts.append(uvt)
                    for jp in jpairs:
                        nj = njs[jp[0]]
                        sj0 = chunks[jp[0]][0]
                        for i in range(NCHUNK):
                            wi = WSTARTS[i]
                            ps = pspool.tile([WIN, 512], f32, tag="ps")
                            for t, j in enumerate(jp):
                                xh = xhls[j][:, 0, wi : wi + WIN, ci]
                                xl = xhls[j][:, 1, wi : wi + WIN, ci]
                                sl = ps[:, t * 2 * nj : (t + 1) * 2 * nj]
                                nc.tensor.matmul(sl, xh, band[f"b1h_{j}"][:], start=True, stop=False)
                                nc.tensor.matmul(sl, xh, band[f"b1l_{j}"][:], start=False, stop=False)
                                nc.tensor.matmul(sl, xl, band[f"b1h_{j}"][:], start=False, stop=True)
                            # drain: hi = fp16(psum) then lo = fp16(psum - hi)
                            if len(jp) > 1:
                                src = ps[:, 0 : len(jp) * 2 * nj].rearrange(
                                    "m (js uv x) -> m uv js x", js=len(jp), uv=2
                                )
                                dhi = uvts[i][:, :, 0, sj0 : sj0 + len(jp) * nj].rearrange(
                                    "m uv (js x) -> m uv js x", js=len(jp)
                                )
                                dlo = uvts[i][:, :, 1, sj0 : sj0 + len(jp) * nj].rearrange(
                                    "m uv (js x) -> m uv js x", js=len(jp)
                                )
                            else:
                                src = ps[:, 0 : 2 * nj].rearrange("m (uv x) -> m uv x", uv=2)
                                dhi = uvts[i][:, :, 0, sj0 : sj0 + nj]
                                dlo = uvts[i][:, :, 1, sj0 : sj0 + nj]
                            if i % 2 == 0:
                                nc.vector.tensor_copy(dhi, src)
                            else:
                                nc.scalar.copy(dhi, src)
                            nc.vector.tensor_sub(dlo, src, dhi)
                    # stage 2: x-conv back to image orientation + clip
                    for b in range(n_yblk):
                        pso = psopool.tile([128, 512], f32, tag="pso")
                        for j in range(NCHUNK):
                            s, e, _ = chunks[j]
                            sl = pso[:, s:e]
                            yb = slice(b * 128, (b + 1) * 128)
                            uh = uvts[j][:, 0, 0, yb]
                            ul = uvts[j][:, 0, 1, yb]
                            vh = uvts[j][:, 1, 0, yb]
                            vl = uvts[j][:, 1, 1, yb]
                            nc.tensor.matmul(sl, uh, band[f"buh_{j}"][:], start=True, stop=False)
                            nc.tensor.matmul(sl, uh, band[f"bul_{j}"][:], start=False, stop=False)
                            nc.tensor.matmul(sl, ul, band[f"buh_{j}"][:], start=False, stop=False)
                            nc.tensor.matmul(sl, vh, band[f"bvh_{j}"][:], start=False, stop=False)
                            nc.tensor.matmul(sl, vh, band[f"bvl_{j}"][:], start=False, stop=False)
                            nc.tensor.matmul(sl, vl, band[f"bvh_{j}"][:], start=False, stop=True)
                        dst = outs[b][:, :, ci]
                        nc.scalar.activation(dst, pso[:], relu, bias=1.0)
                        nc.vector.tensor_scalar_min(dst, dst, 255.0)
                for b in range(n_yblk):
                    nc.sync.dma_start(
                        out_d.ap()[n, b * 128 : (b + 1) * 128, :, :], outs[b][:]
                    )

    nc.compile()
    return nc


_CACHE = {}


def _get_nc():
    if "nc" not in _CACHE:
        _CACHE["nc"] = build_bass()
    return _CACHE["nc"]


def kernel(x: np.ndarray) -> np.ndarray:
    from concourse import bass_utils

    nc = _get_nc()
    b1h, b1l = make_bands1()
    b2 = make_bands2()
    const_map = {}
    for j in range(NCHUNK):
        const_map[f"b1h_{j}"] = b1h[j]
        const_map[f"b1l_{j}"] = b1l[j]
        for k in ("buh", "bul", "bvh", "bvl"):
            const_map[f"{k}_{j}"] = b2[k][j]
    x = np.ascontiguousarray(x, dtype=np.float32)
    in_maps = [
        {"x": x[k * IMG_PER_CORE : (k + 1) * IMG_PER_CORE], **const_map}
        for k in range(N_CORES)
    ]
    res = bass_utils.run_bass_kernel_spmd(nc, in_maps, core_ids=list(range(N_CORES)))
    _CACHE["last_result"] = res
    out = np.concatenate([r["out"] for r in res.results], axis=0)
    return out.astype(np.float32)


# revision 5
# speedup vs baseline: 1.3831x; 1.0291x over previous
"""LoG kernel v4: fp16 hi/lo compensated split -> all matmuls at 1 cycle/row.

v3 (fp32 matmuls, 4 cyc/row) is Tensor-bound at ~226 us PE busy. fp16 MMs
stream 1 cyc/row at any N and get FWL (4x faster LDWEIGHTS), but naive fp16
fails: the composed filter's gain (sqrt(sum k^2) ~ 2563) amplifies operand
rounding. Fix: first-order error-compensated splits of BOTH the data and the
bands:  p*q ~= p_hi*q_hi + p_hi*q_lo + p_lo*q_hi  (p_lo*q_lo dropped).
  stage 1: 3 MMs per (x-window, y-chunk):  xh*b1h, xh*b1l, xl*b1h
  stage 2: 6 MMs per (y-blk, x-chunk):     uh*Bh, uh*Bl, ul*Bh, vh*Ah, vh*Al, vl*Ah
Splits: x_hi=fp16(x), x_lo=fp16(x-x_hi) computed on ScalarE/DVE per image;
u_hi=fp16(psum), u_lo=fp16(psum-u_hi) in the stage-1 drain (2 passes).
Residual error ~0.3 rms (fp16 max 65504 just fits u <= 255*256 = 65280).

Geometry + DMA rules identical to v3 (five 128-row windows per axis; all
DMAs exactly 128 partitions; chunk psum slices disjoint).
"""

import numpy as np

N_CORES = 8
BATCH = 32
IMG_PER_CORE = BATCH // N_CORES
H = W = 512
C = 3
RAD = 5
WIN = 128

WSTARTS = [0, 93, 191, 289, 384]
CBOUNDS = [0, 98, 196, 294, 392, 512]
NCHUNK = 5


def _chunks():
    return [(CBOUNDS[j], CBOUNDS[j + 1], WSTARTS[j]) for j in range(NCHUNK)]


def make_taps():
    g = np.exp(-((np.arange(3) - 1.0) ** 2) / 2.0)
    g = g / g.sum()
    S = np.array([1, 8, 28, 56, 70, 56, 28, 8, 1], dtype=np.float64)
    D2 = np.array([1, 4, 4, -4, -10, -4, 4, 4, 1], dtype=np.float64)
    return np.convolve(g, S), np.convolve(g, D2)


def _reflect(i, n):
    if i < 0:
        return -i
    if i > n - 1:
        return 2 * (n - 1) - i
    return i


def _split16(a):
    hi = a.astype(np.float16)
    lo = (a - hi.astype(np.float64)).astype(np.float16)
    return hi, lo


def make_bands1():
    """Per chunk j: [128, 2nj] = [A_j | B_j], split into fp16 hi/lo."""
    A, B = make_taps()
    his, los = [], []
    for s, e, w0 in _chunks():
        nj = e - s
        band = np.zeros((WIN, 2 * nj), np.float64)
        for t in range(nj):
            for off in range(-RAD, RAD + 1):
                r = _reflect(s + t + off, H) - w0
                band[r, t] += A[off + RAD]
                band[r, nj + t] += B[off + RAD]
        hi, lo = _split16(band)
        his.append(hi)
        los.append(lo)
    return his, los


def make_bands2():
    """Per chunk j: [128, nj] B-band (for u) and A-band (for v), fp16 hi/lo."""
    A, B = make_taps()
    out = {"buh": [], "bul": [], "bvh": [], "bvl": []}
    for s, e, w0 in _chunks():
        nj = e - s
        tb = np.zeros((WIN, nj), np.float64)
        ta = np.zeros((WIN, nj), np.float64)
        for t in range(nj):
            for off in range(-RAD, RAD + 1):
                r = _reflect(s + t + off, W) - w0
                tb[r, t] += B[off + RAD]
                ta[r, t] += A[off + RAD]
        bh, bl = _split16(tb)
        ah, al = _split16(ta)
        out["buh"].append(bh)
        out["bul"].append(bl)
        out["bvh"].append(ah)
        out["bvl"].append(al)
    return out


def _pairs(n):
    ps, i = [], 0
    while i < n:
        ps.append(tuple(range(i, min(i + 2, n))))
        i += 2
    return ps


def build_bass(n_imgs=IMG_PER_CORE, h=H, w=W, c=C):
    import concourse.bacc as bacc
    import concourse.mybir as mybir
    import concourse.tile as tile

    f32 = mybir.dt.float32
    f16 = mybir.dt.float16
    relu = mybir.ActivationFunctionType.Relu
    chunks = _chunks()
    jpairs = _pairs(NCHUNK)
    n_yblk = h // 128

    nc = bacc.Bacc("TRN2", target_bir_lowering=False, debug=False)
    x_d = nc.dram_tensor("x", [n_imgs, h, w, c], f32, kind="ExternalInput")
    out_d = nc.dram_tensor("out", [n_imgs, h, w, c], f32, kind="ExternalOutput")
    njs = [e - s for s, e, _ in chunks]
    band_names = (
        [(f"b1h_{j}", 2 * njs[j]) for j in range(NCHUNK)]
        + [(f"b1l_{j}", 2 * njs[j]) for j in range(NCHUNK)]
        + [(f"{k}_{j}", njs[j]) for k in ("buh", "bul", "bvh", "bvl") for j in range(NCHUNK)]
    )
    band_d = {
        name: nc.dram_tensor(name, [WIN, width], f16, kind="ExternalInput")
        for name, width in band_names
    }

    with tile.TileContext(nc) as tc:
        with (
            tc.tile_pool(name="const", bufs=1) as cpool,
            tc.tile_pool(name="xin", bufs=2) as xpool,
            tc.tile_pool(name="xhl", bufs=2) as xhlpool,
            tc.tile_pool(name="uv", bufs=2) as uvpool,
            tc.tile_pool(name="outp", bufs=2) as opool,
            tc.tile_pool(name="ps", bufs=4, space="PSUM") as pspool,
            tc.tile_pool(name="pso", bufs=4, space="PSUM") as psopool,
        ):
            band = {}
            for name, width in band_names:
                tb = cpool.tile([WIN, width], f16, name=name)
                nc.sync.dma_start(tb[:], band_d[name].ap())
                band[name] = tb

            for n in range(n_imgs):
                xhls = []
                for j in range(NCHUNK):
                    w0 = WSTARTS[j]
                    xr = xpool.tile([WIN, w, c], f32, tag=f"xf{j % 2}", name=f"x{j}_{n}")
                    nc.sync.dma_start(xr[:], x_d.ap()[n, w0 : w0 + WIN, :, :])
                    xhl = xhlpool.tile([WIN, 2, w, c], f16, tag=f"xhl{j}", name=f"xhl{j}_{n}")
                    # split: hi = fp16(x) on ScalarE, lo = fp16(x - hi) on DVE
                    nc.scalar.copy(xhl[:, 0], xr[:])
                    nc.vector.tensor_sub(xhl[:, 1], xr[:], xhl[:, 0])
                    xhls.append(xhl)
                outs = []
                for b in range(n_yblk):
                    ot = opool.tile([128, w, c], f32, tag=f"o{b}", name=f"o{b}_{n}")
                    outs.append(ot)
                for ci in range(c):
                    # stage 1: y-conv, transposed output per x-window
                    uvts = []
                    for i in range(NCHUNK):
                        uvt = uvpool.tile(
                            [WIN, 2, 2, h], f16, tag=f"uv{i}", name=f"uv{i}_{n}_{ci}"
                        )
                        uvts.append(uvt)
                    for jp in jpairs:
                        nj = njs[jp[0]]
                        sj0 = chunks[jp[0]][0]
                        for i in range(NCHUNK):
                            wi = WSTARTS[i]
                            ps = pspool.tile([WIN, 512], f32, tag="ps")
                            for t, j in enumerate(jp):
                                xh = xhls[j][:, 0, wi : wi + WIN, ci]
                                xl = xhls[j][:, 1, wi : wi + WIN, ci]
                                sl = ps[:, t * 2 * nj : (t + 1) * 2 * nj]
                                nc.tensor.matmul(sl, xh, band[f"b1h_{j}"][:], start=True, stop=False)
                                nc.tensor.matmul(sl, xh, band[f"b1l_{j}"][:], start=False, stop=False)
                                nc.tensor.matmul(sl, xl, band[f"b1h_{j}"][:], start=False, stop=True)
                            # drain: hi = fp16(psum) then lo = fp16(psum - hi)
                            if len(jp) > 1:
                                src = ps[:, 0 : len(jp) * 2 * nj].rearrange(
                                    "m (js uv x) -> m uv js x", js=len(jp), uv=2
                                )
                                dhi = uvts[i][:, :, 0, sj0 : sj0 + len(jp) * nj].rearrange(
                                    "m uv (js x) -> m uv js x", js=len(jp)
                                )
                                dlo = uvts[i][:, :, 1, sj0 : sj0 + len(jp) * nj].rearrange(
                                    "m uv (js x) -> m uv js x", js=len(jp)
                                )
                            else:
                                src = ps[:, 0 : 2 * nj].rearrange("m (uv x) -> m uv x", uv=2)
                                dhi = uvts[i][:, :, 0, sj0 : sj0 + nj]
                                dlo = uvts[i][:, :, 1, sj0 : sj0 + nj]
                            if i in (0, 4):
                                nc.vector.tensor_copy(dhi, src)
                            else:
                                nc.scalar.copy(dhi, src)
                            nc.vector.tensor_sub(dlo, src, dhi)
                    # stage 2: x-conv back to image orientation + clip
                    for b in range(n_yblk):
                        pso = psopool.tile([128, 512], f32, tag="pso")
                        for j in range(NCHUNK):
                            s, e, _ = chunks[j]
                            sl = pso[:, s:e]
                            yb = slice(b * 128, (b + 1) * 128)
                            uh = uvts[j][:, 0, 0, yb]
                            ul = uvts[j][:, 0, 1, yb]
                            vh = uvts[j][:, 1, 0, yb]
                            vl = uvts[j][:, 1, 1, yb]
                            nc.tensor.matmul(sl, uh, band[f"buh_{j}"][:], start=True, stop=False)
                            nc.tensor.matmul(sl, uh, band[f"bul_{j}"][:], start=False, stop=False)
                            nc.tensor.matmul(sl, ul, band[f"buh_{j}"][:], start=False, stop=False)
                            nc.tensor.matmul(sl, vh, band[f"bvh_{j}"][:], start=False, stop=False)
                            nc.tensor.matmul(sl, vh, band[f"bvl_{j}"][:], start=False, stop=False)
                            nc.tensor.matmul(sl, vl, band[f"bvh_{j}"][:], start=False, stop=True)
                        dst = outs[b][:, :, ci]
                        nc.scalar.activation(dst, pso[:], relu, bias=1.0)
                        nc.vector.tensor_scalar_min(dst, dst, 255.0)
                for b in range(n_yblk):
                    nc.sync.dma_start(
                        out_d.ap()[n, b * 128 : (b + 1) * 128, :, :], outs[b][:]
                    )

    nc.compile()
    return nc


_CACHE = {}


def _get_nc():
    if "nc" not in _CACHE:
        _CACHE["nc"] = build_bass()
    return _CACHE["nc"]


def kernel(x: np.ndarray) -> np.ndarray:
    from concourse import bass_utils

    nc = _get_nc()
    b1h, b1l = make_bands1()
    b2 = make_bands2()
    const_map = {}
    for j in range(NCHUNK):
        const_map[f"b1h_{j}"] = b1h[j]
        const_map[f"b1l_{j}"] = b1l[j]
        for k in ("buh", "bul", "bvh", "bvl"):
            const_map[f"{k}_{j}"] = b2[k][j]
    x = np.ascontiguousarray(x, dtype=np.float32)
    in_maps = [
        {"x": x[k * IMG_PER_CORE : (k + 1) * IMG_PER_CORE], **const_map}
        for k in range(N_CORES)
    ]
    res = bass_utils.run_bass_kernel_spmd(nc, in_maps, core_ids=list(range(N_CORES)))
    _CACHE["last_result"] = res
    out = np.concatenate([r["out"] for r in res.results], axis=0)
    return out.astype(np.float32)
